# revision 3
# baseline (speedup 1.0000x reference)
"""GAT+GCN Trainium2 kernel: 8-core SPMD Bass/Tile implementation.

Sharding: nodes/graphs split contiguously across cores.  Edges assigned
to the core owning their dst node, sorted by dst, padded per 128-node
dst tile to K chunks of 128 edges.  Gather tables (h|a_src, g1) are
fp16 and AllGathered across cores.
"""
import numpy as np
import concourse.bass as bass
import concourse.bacc as bacc
import concourse.mybir as mybir
import concourse.tile as tile

f32 = mybir.dt.float32
f16 = mybir.dt.float16
i32 = mybir.dt.int32
AF = mybir.ActivationFunctionType
OP = mybir.AluOpType
AX = mybir.AxisListType

F = 78          # input feature dim
H = 10          # heads
HID = 780       # F*H
FW = HID + H    # htab row: h | a_src


def ceil_div(a, b):
    return (a + b - 1) // b


def host_prep(inp, n_cores=8):
    """Build per-core input maps + cfg from full inputs."""
    x = np.ascontiguousarray(np.asarray(inp["x"], np.float32))
    ei = np.asarray(inp["edge_index"], np.int64)
    tgt = np.asarray(inp["target"], np.int64)
    N = x.shape[0]
    B = tgt.shape[0]
    GN = N // B                # nodes per graph
    NS = N // n_cores
    T = NS // 128
    BL = B // n_cores

    loops = np.arange(N, dtype=np.int64)
    src = np.concatenate([ei[0], loops])
    dst = np.concatenate([ei[1], loops])
    E = src.shape[0]

    deg = np.bincount(dst, minlength=N).astype(np.float64)
    dinv = 1.0 / np.sqrt(deg)
    normv = (dinv[src] * dinv[dst]).astype(np.float32)

    order = np.argsort(dst, kind="stable")
    src_s = src[order].astype(np.int32)
    dst_s = dst[order].astype(np.int32)
    norm_s = normv[order]

    gtile = dst_s // 128
    n_gtiles = N // 128
    starts = np.searchsorted(gtile, np.arange(n_gtiles))
    cnts = np.searchsorted(gtile, np.arange(n_gtiles), side="right") - starts
    K = int(np.max(ceil_div(cnts, 128)))

    srcs_p = np.zeros((n_gtiles, 128, K), np.int32)
    dstf_p = np.full((n_gtiles, 128, K), 128.0, np.float32)
    norm_p = np.zeros((n_gtiles, 128, K), np.float32)
    j = np.arange(E) - starts[gtile]
    srcs_p[gtile, j % 128, j // 128] = src_s
    dstf_p[gtile, j % 128, j // 128] = (dst_s % 128).astype(np.float32)
    norm_p[gtile, j % 128, j // 128] = norm_s

    gat_w = np.asarray(inp["gat_w"], np.float32)
    att_src = np.asarray(inp["att_src"], np.float32)
    att_dst = np.asarray(inp["att_dst"], np.float32)
    As = np.einsum("fhc,hc->fh", gat_w.reshape(F, H, F), att_src)
    Ad = np.einsum("fhc,hc->fh", gat_w.reshape(F, H, F), att_dst)
    asad = np.concatenate([As, Ad], 1).astype(np.float32)
    gatb_rep = np.broadcast_to(np.asarray(inp["gat_b"], np.float32), (128, HID)).copy()
    gcn_w = np.asarray(inp["gcn_w"], np.float32)
    gcnb_rep = np.broadcast_to(np.asarray(inp["gcn_b"], np.float32), (128, HID)).copy()

    fcg1_w = np.asarray(inp["fcg1_w"], np.float32).copy()
    fcg1_w[HID:] *= 1.0 / GN

    def bias_sw(b, mt):
        b = np.asarray(b, np.float32)
        out = np.zeros((mt * 128,), np.float32)
        out[: b.shape[0]] = b
        return np.ascontiguousarray(out.reshape(mt, 128).T)

    fcg1_bsw = bias_sw(inp["fcg1_b"], 12)
    fcg2_w = np.asarray(inp["fcg2_w"], np.float32)
    fcg2_bsw = bias_sw(inp["fcg2_b"], 1)

    convxt_w = np.asarray(inp["convxt_w"], np.float32)
    W2 = np.ascontiguousarray(convxt_w.transpose(1, 2, 0).reshape(1000, 8 * 32))
    emb = np.asarray(inp["emb"], np.float32)
    fcxt_w = np.asarray(inp["fcxt_w"], np.float32)
    cb = np.asarray(inp["convxt_b"], np.float32)
    bias_fold = (cb[:, None] * fcxt_w.reshape(32, 121, 128).sum(1)).sum(0)
    fcxt_bsw = bias_sw(np.asarray(inp["fcxt_b"], np.float32) + bias_fold, 1)

    fc1_w = np.asarray(inp["fc1_w"], np.float32)
    fc1_bsw = bias_sw(inp["fc1_b"], 8)
    fc2_w = np.asarray(inp["fc2_w"], np.float32)
    fc2_bsw = bias_sw(inp["fc2_b"], 4)
    out_w = np.asarray(inp["out_w"], np.float32)
    out_b = np.asarray(inp["out_b"], np.float32).reshape(1, 1)

    iota_rep = np.broadcast_to(
        np.tile(np.arange(128, dtype=np.float32), K), (128, K * 128)).copy()
    iota26 = np.broadcast_to(
        np.tile(np.arange(26, dtype=np.float32), 8), (128, 8 * 26)).copy()
    ident = np.eye(128, dtype=np.float32)
    ident16 = np.eye(128, dtype=np.float16)
    ng = 128 // GN
    poolm = np.zeros((128, ng), np.float32)
    for g in range(ng):
        poolm[g * GN:(g + 1) * GN, g] = 1.0

    shared = dict(
        gat_w=gat_w, asad=asad, gatb_rep=gatb_rep, gcn_w=gcn_w,
        gcnb_rep=gcnb_rep, fcg1_w=fcg1_w, fcg1_bsw=fcg1_bsw, fcg2_w=fcg2_w,
        fcg2_bsw=fcg2_bsw, w2=W2, emb=emb, fcxt_w=fcxt_w, fcxt_bsw=fcxt_bsw,
        fc1_w=fc1_w, fc1_bsw=fc1_bsw, fc2_w=fc2_w, fc2_bsw=fc2_bsw,
        out_w=out_w, out_b=out_b, iota_rep=iota_rep, iota26=iota26,
        ident=ident, ident16=ident16, poolm=poolm,
    )

    in_maps = []
    for c in range(n_cores):
        m = dict(shared)
        m["x_sl"] = np.ascontiguousarray(x[c * NS:(c + 1) * NS])
        gt = slice(c * T, (c + 1) * T)
        m["srcs"] = np.ascontiguousarray(srcs_p[gt].reshape(T * 128, K))
        m["dstf"] = np.ascontiguousarray(dstf_p[gt].reshape(T * 128, K))
        m["normv"] = np.ascontiguousarray(norm_p[gt].reshape(T * 128, K))
        tpad = np.zeros((BL, 1024), np.int64)
        tpad[:, :1000] = tgt[c * BL:(c + 1) * BL]
        tl = tpad.reshape(BL, 8, 128)
        m["t_sb"] = np.ascontiguousarray(
            tl.transpose(2, 0, 1).reshape(128, BL * 8).astype(np.float32))
        in_maps.append(m)

    cfg = dict(n_cores=n_cores, N=N, NS=NS, T=T, BL=BL, K=K, GN=GN)
    return in_maps, cfg


def build(cfg):
    n_cores, NS, T, BL, K, GN = (cfg["n_cores"], cfg["NS"], cfg["T"],
                                 cfg["BL"], cfg["K"], cfg["GN"])
    N = cfg["N"]

    nc = bacc.Bacc(None, target_bir_lowering=False)

    def dinp(name, shape, dt=f32):
        return nc.dram_tensor(name, list(shape), dt, kind="ExternalInput")

    x_sl = dinp("x_sl", (NS, F))
    srcs = dinp("srcs", (T * 128, K), i32)
    dstf = dinp("dstf", (T * 128, K))
    normv = dinp("normv", (T * 128, K))
    t_sb_d = dinp("t_sb", (128, BL * 8))
    gat_w = dinp("gat_w", (F, HID))
    asad = dinp("asad", (F, 2 * H))
    gatb_rep = dinp("gatb_rep", (128, HID))
    gcn_w = dinp("gcn_w", (HID, HID))
    gcnb_rep = dinp("gcnb_rep", (128, HID))
    fcg1_w = dinp("fcg1_w", (2 * HID, 1500))
    fcg1_bsw = dinp("fcg1_bsw", (128, 12))
    fcg2_w = dinp("fcg2_w", (1500, 128))
    fcg2_bsw = dinp("fcg2_bsw", (128, 1))
    w2_d = dinp("w2", (1000, 256))
    emb_d = dinp("emb", (26, 128))
    fcxt_w = dinp("fcxt_w", (32 * 121, 128))
    fcxt_bsw = dinp("fcxt_bsw", (128, 1))
    fc1_w = dinp("fc1_w", (256, 1024))
    fc1_bsw = dinp("fc1_bsw", (128, 8))
    fc2_w = dinp("fc2_w", (1024, 512))
    fc2_bsw = dinp("fc2_bsw", (128, 4))
    out_w = dinp("out_w", (512, 1))
    out_b = dinp("out_b", (1, 1))
    iota_rep = dinp("iota_rep", (128, K * 128))
    iota26 = dinp("iota26", (128, 8 * 26))
    ident = dinp("ident", (128, 128))
    ident16 = dinp("ident16", (128, 128), f16)
    poolm = dinp("poolm", (128, 128 // GN))

    outp = nc.dram_tensor("outp", [BL, 1], f32, kind="ExternalOutput")

    htab_sl = nc.dram_tensor("htab_sl", [NS, FW], f16)
    htab = nc.dram_tensor("htab", [N, FW], f16, addr_space="Shared")
    g1_sl = nc.dram_tensor("g1_sl", [NS, HID], f16)
    g1tab = nc.dram_tensor("g1tab", [N, HID], f16, addr_space="Shared")

    FCH = [(kk * 128, min(128, HID - kk * 128)) for kk in range(ceil_div(HID, 128))]

    def tiles(n, step=128):
        return [(s, min(step, n - s)) for s in range(0, n, step)]

    with tile.TileContext(nc) as tc:
        with (
            tc.tile_pool(name="const", bufs=1) as cpool,
            tc.tile_pool(name="sb", bufs=2) as pool,
            tc.tile_pool(name="w", bufs=3) as wpool,
            tc.tile_pool(name="ps", bufs=2, space="PSUM") as psp,
            tc.tile_pool(name="pstr", bufs=2, space="PSUM") as pst,
            tc.tile_pool(name="psg", bufs=1, space="PSUM") as psg,
            tc.tile_pool(name="psm", bufs=1, space="PSUM") as psm,
        ):
            # ---------- resident constants ----------
            def load_const(name, dram, shape, dt=f32, rows=None):
                t_ = cpool.tile(list(shape), dt, tag=name)
                if rows is None:
                    nc.sync.dma_start(out=t_[:], in_=dram[:])
                else:
                    nc.sync.dma_start(out=t_[:rows, :], in_=dram[:])
                return t_

            gatw_sb = load_const("gatw", gat_w, [F, HID])
            asad_sb = load_const("asad", asad, [F, 2 * H])
            gatb_sb = load_const("gatb", gatb_rep, [128, HID])
            gcnb_sb = load_const("gcnb", gcnb_rep, [128, HID])
            iota_sb = load_const("iota", iota_rep, [128, K * 128])
            iota26_sb = load_const("iota26", iota26, [128, 8 * 26])
            ident_sb = load_const("ident", ident, [128, 128])
            ident16_sb = load_const("ident16", ident16, [128, 128], f16)
            poolm_sb = load_const("poolm", poolm, [128, 2])
            emb_sb = load_const("emb", emb_d, [26, 128])
            t_sb = load_const("tsb", t_sb_d, [128, BL * 8])
            fcg1b_sb = load_const("fcg1b", fcg1_bsw, [128, 12])
            fcg2b_sb = load_const("fcg2b", fcg2_bsw, [128, 1])
            fcxtb_sb = load_const("fcxtb", fcxt_bsw, [128, 1])
            fc1b_sb = load_const("fc1b", fc1_bsw, [128, 8])
            fc2b_sb = load_const("fc2b", fc2_bsw, [128, 4])
            outb_sb = load_const("outb", out_b, [1, 1])

            gcnw_sb = []
            for kk, (ks, kn) in enumerate(FCH):
                t_ = cpool.tile([128, HID], f32, tag=f"gcnw{kk}")
                nc.sync.dma_start(out=t_[:kn, :], in_=gcn_w[ks:ks + kn, :])
                gcnw_sb.append(t_)
            w2_sb = []
            for ic in range(8):
                icn = min(128, 1000 - ic * 128)
                t_ = cpool.tile([128, 256], f32, tag=f"w2{ic}")
                nc.sync.dma_start(out=t_[:icn, :], in_=w2_d[ic * 128:ic * 128 + icn, :])
                w2_sb.append(t_)
            adst_sb = cpool.tile([128, T * H], f16, tag="adst")

            # ================= Phase A: h | a_src | a_dst ===============
            for t in range(T):
                rows = slice(t * 128, (t + 1) * 128)
                x_t = pool.tile([128, F], f32, tag="x_t")
                nc.sync.dma_start(out=x_t[:], in_=x_sl[rows, :])
                xt_ps = pst.tile([128, 128], f32, tag="tr")
                nc.tensor.transpose(out=xt_ps[:F, :], in_=x_t[:], identity=ident_sb[:])
                xT = pool.tile([F, 128], f32, tag="xT")
                nc.scalar.activation(out=xT[:], in_=xt_ps[:F, :], func=AF.Copy)
                h_ps = psp.tile([128, HID], f32, tag="big")
                nc.tensor.matmul(out=h_ps[:, :512], lhsT=xT[:], rhs=gatw_sb[:, :512],
                                 start=True, stop=True)
                nc.tensor.matmul(out=h_ps[:, 512:], lhsT=xT[:], rhs=gatw_sb[:, 512:],
                                 start=True, stop=True)
                asd_ps = psp.tile([128, 96], f32, tag="small")
                nc.tensor.matmul(out=asd_ps[:, :2 * H], lhsT=xT[:], rhs=asad_sb[:],
                                 start=True, stop=True)
                htile = pool.tile([128, HID], f16, tag="htile")
                nc.scalar.activation(out=htile[:], in_=h_ps[:], func=AF.Copy)
                asd16 = pool.tile([128, 2 * H], f16, tag="asd")
                nc.vector.tensor_copy(out=asd16[:], in_=asd_ps[:, :2 * H])
                nc.sync.dma_start(out=htab_sl[rows, :HID], in_=htile[:])
                nc.sync.dma_start(out=htab_sl[rows, HID:FW], in_=asd16[:, :H])
                nc.vector.tensor_copy(out=adst_sb[:, t * H:(t + 1) * H],
                                      in_=asd16[:, H:])

            nc.gpsimd.collective_compute(
                "AllGather", OP.bypass,
                replica_groups=[list(range(n_cores))],
                ins=[htab_sl[:]], outs=[htab[:]],
            )

            # ================= Phase B: GAT edge aggregation =============
            for t in range(T):
                rows = slice(t * 128, (t + 1) * 128)
                sc = pool.tile([128, K], i32, tag="sc")
                nc.sync.dma_start(out=sc[:], in_=srcs[rows, :])
                df = pool.tile([128, K], f32, tag="df")
                nc.sync.dma_start(out=df[:], in_=dstf[rows, :])
                G = pool.tile([128, K * FW], f16, tag="G")
                for c in range(K):
                    nc.gpsimd.indirect_dma_start(
                        out=G[:, c * FW:(c + 1) * FW], out_offset=None,
                        in_=htab[:],
                        in_offset=bass.IndirectOffsetOnAxis(ap=sc[:, c:c + 1], axis=0),
                    )
                sel = pool.tile([128, K * 128], f16, tag="sel")
                nc.vector.tensor_tensor(
                    out=sel[:].rearrange("p (k d) -> p k d", d=128),
                    in0=iota_sb[:].rearrange("p (k d) -> p k d", d=128),
                    in1=df[:].unsqueeze(2).to_broadcast([128, K, 128]),
                    op=OP.is_equal)
                sm_ps = psp.tile([128, 96], f32, tag="small")
                for c in range(K):
                    st_ps = pst.tile([128, 128], f16, tag="tr16")
                    nc.tensor.transpose(out=st_ps[:], in_=sel[:, c * 128:(c + 1) * 128],
                                        identity=ident16_sb[:])
                    selT = pool.tile([128, 128], f16, tag=f"selT{c}")
                    nc.scalar.activation(out=selT[:], in_=st_ps[:], func=AF.Copy)
                    nc.tensor.matmul(out=sm_ps[:, c * H:(c + 1) * H], lhsT=selT[:],
                                     rhs=adst_sb[:, t * H:(t + 1) * H],
                                     start=True, stop=True)
                al = pool.tile([128, K * H], f32, tag="al")
                nc.vector.tensor_tensor(
                    out=al[:].rearrange("p (k h) -> p k h", h=H),
                    in0=G[:].rearrange("p (k w) -> p k w", w=FW)[:, :, HID:FW],
                    in1=sm_ps[:, :K * H].rearrange("p (k h) -> p k h", h=H),
                    op=OP.add)
                al2 = pool.tile([128, K * H], f32, tag="al2")
                nc.vector.tensor_scalar(out=al2[:], in0=al[:], scalar1=0.2,
                                        scalar2=None, op0=OP.mult)
                nc.vector.tensor_tensor(out=al2[:], in0=al2[:], in1=al[:], op=OP.max)
                p16 = pool.tile([128, K * H], f16, tag="p16")
                nc.scalar.activation(out=p16[:], in_=al2[:], func=AF.Exp)
                for c in range(K):
                    nc.tensor.matmul(out=sm_ps[:, 80:80 + H],
                                     lhsT=sel[:, c * 128:(c + 1) * 128],
                                     rhs=p16[:, c * H:(c + 1) * H],
                                     start=(c == 0), stop=(c == K - 1))
                m = pool.tile([128, K * HID], f16, tag="m")
                nc.vector.tensor_tensor(
                    out=m[:].rearrange("p (k h r) -> p k h r", h=H, r=F),
                    in0=G[:].rearrange("p (k w) -> p k w", w=FW)[:, :, :HID]
                         .rearrange("p k (h r) -> p k h r", r=F),
                    in1=p16[:].rearrange("p (k h) -> p k h", h=H)
                         .unsqueeze(3).to_broadcast([128, K, H, F]),
                    op=OP.mult)
                g1_ps = psp.tile([128, HID], f32, tag="big")
                for c in range(K):
                    nc.tensor.matmul(out=g1_ps[:, :512],
                                     lhsT=sel[:, c * 128:(c + 1) * 128],
                                     rhs=m[:, c * HID: c * HID + 512],
                                     start=(c == 0), stop=(c == K - 1))
                    nc.tensor.matmul(out=g1_ps[:, 512:],
                                     lhsT=sel[:, c * 128:(c + 1) * 128],
                                     rhs=m[:, c * HID + 512:(c + 1) * HID],
                                     start=(c == 0), stop=(c == K - 1))
                rd = pool.tile([128, H], f32, tag="rd")
                nc.vector.reciprocal(out=rd[:], in_=sm_ps[:, 80:80 + H])
                g1n = pool.tile([128, HID], f32, tag="g1n")
                nc.vector.tensor_tensor(
                    out=g1n[:].rearrange("p (h r) -> p h r", r=F),
                    in0=g1_ps[:].rearrange("p (h r) -> p h r", r=F),
                    in1=rd[:].unsqueeze(2).to_broadcast([128, H, F]),
                    op=OP.mult)
                nc.vector.tensor_tensor(out=g1n[:], in0=g1n[:], in1=gatb_sb[:],
                                        op=OP.add)
                g1t = pool.tile([128, HID], f16, tag="g1t")
                nc.scalar.activation(out=g1t[:], in_=g1n[:], func=AF.Relu)
                nc.sync.dma_start(out=g1_sl[rows, :], in_=g1t[:])

            nc.gpsimd.collective_compute(
                "AllGather", OP.bypass,
                replica_groups=[list(range(n_cores))],
                ins=[g1_sl[:]], outs=[g1tab[:]],
            )

            # ============ Phase D: GCN aggregation + z + pooling =========
            gap_ps = psg.tile([BL, HID], f32, tag="gap")
            gmpT_sb = []
            gapT_sb = []
            for kk, (ks, kn) in enumerate(FCH):
                gmpT_sb.append(cpool.tile([128, BL], f32, tag=f"gmpT{kk}"))
                gapT_sb.append(cpool.tile([128, BL], f32, tag=f"gapT{kk}"))

            for t in range(T):
                rows = slice(t * 128, (t + 1) * 128)
                sc = pool.tile([128, K], i32, tag="sc")
                nc.sync.dma_start(out=sc[:], in_=srcs[rows, :])
                df = pool.tile([128, K], f32, tag="df")
                nc.sync.dma_start(out=df[:], in_=dstf[rows, :])
                nv = pool.tile([128, K], f32, tag="nv")
                nc.sync.dma_start(out=nv[:], in_=normv[rows, :])
                G2 = pool.tile([128, K * HID], f16, tag="G")
                for c in range(K):
                    nc.gpsimd.indirect_dma_start(
                        out=G2[:, c * HID:(c + 1) * HID], out_offset=None,
                        in_=g1tab[:],
                        in_offset=bass.IndirectOffsetOnAxis(ap=sc[:, c:c + 1], axis=0),
                    )
                sel = pool.tile([128, K * 128], f16, tag="sel")
                nc.vector.tensor_tensor(
                    out=sel[:].rearrange("p (k d) -> p k d", d=128),
                    in0=iota_sb[:].rearrange("p (k d) -> p k d", d=128),
                    in1=df[:].unsqueeze(2).to_broadcast([128, K, 128]),
                    op=OP.is_equal)
                wsel = pool.tile([128, K * 128], f16, tag="wsel")
                nc.vector.tensor_tensor(
                    out=wsel[:].rearrange("p (k d) -> p k d", d=128),
                    in0=sel[:].rearrange("p (k d) -> p k d", d=128),
                    in1=nv[:].unsqueeze(2).to_broadcast([128, K, 128]),
                    op=OP.mult)
                agg_ps = psp.tile([128, HID], f32, tag="big")
                for c in range(K):
                    nc.tensor.matmul(out=agg_ps[:, :512],
                                     lhsT=wsel[:, c * 128:(c + 1) * 128],
                                     rhs=G2[:, c * HID: c * HID + 512],
                                     start=(c == 0), stop=(c == K - 1))
                    nc.tensor.matmul(out=agg_ps[:, 512:],
                                     lhsT=wsel[:, c * 128:(c + 1) * 128],
                                     rhs=G2[:, c * HID + 512:(c + 1) * HID],
                                     start=(c == 0), stop=(c == K - 1))
                s_sb = pool.tile([128, HID], f32, tag="s_sb")
                nc.scalar.activation(out=s_sb[:], in_=agg_ps[:], func=AF.Copy)
                z_ps = psp.tile([128, HID], f32, tag="big")
                for kk, (ks, kn) in enumerate(FCH):
                    sT_ps = pst.tile([128, 128], f32, tag="tr")
                    nc.tensor.transpose(out=sT_ps[:kn, :], in_=s_sb[:, ks:ks + kn],
                                        identity=ident_sb[:])
                    sT = pool.tile([128, 128], f32, tag=f"sT{kk}")
                    nc.scalar.activation(out=sT[:kn, :], in_=sT_ps[:kn, :], func=AF.Copy)
                    nc.tensor.matmul(out=z_ps[:, :512], lhsT=sT[:kn, :],
                                     rhs=gcnw_sb[kk][:kn, :512],
                                     start=(kk == 0), stop=(kk == len(FCH) - 1))
                    nc.tensor.matmul(out=z_ps[:, 512:], lhsT=sT[:kn, :],
                                     rhs=gcnw_sb[kk][:kn, 512:],
                                     start=(kk == 0), stop=(kk == len(FCH) - 1))
                g2a = pool.tile([128, HID], f32, tag="g2a")
                nc.vector.tensor_tensor(out=g2a[:], in0=z_ps[:], in1=gcnb_sb[:],
                                        op=OP.add)
                g2b = pool.tile([128, HID], f32, tag="g2b")
                nc.scalar.activation(out=g2b[:], in_=g2a[:], func=AF.Relu)
                ng = 128 // GN      # graphs per tile
                nc.tensor.matmul(out=gap_ps[ng * t:ng * (t + 1), :512],
                                 lhsT=poolm_sb[:, :ng],
                                 rhs=g2b[:, :512], start=True, stop=True)
                nc.tensor.matmul(out=gap_ps[ng * t:ng * (t + 1), 512:],
                                 lhsT=poolm_sb[:, :ng],
                                 rhs=g2b[:, 512:], start=True, stop=True)
                for kk, (ks, kn) in enumerate(FCH):
                    tp_ps = pst.tile([128, 128], f32, tag="tr")
                    nc.tensor.transpose(out=tp_ps[:kn, :], in_=g2b[:, ks:ks + kn],
                                        identity=ident_sb[:])
                    nc.vector.reduce_max(
                        out=gmpT_sb[kk][:kn, ng * t:ng * (t + 1)],
                        in_=tp_ps[:kn, :].rearrange("p (g n) -> p g n", n=GN),
                        axis=AX.X)

            gap_sb = pool.tile([BL, HID], f32, tag="gap_sb")
            nc.scalar.activation(out=gap_sb[:], in_=gap_ps[:], func=AF.Copy)
            for kk, (ks, kn) in enumerate(FCH):
                gp_ps = pst.tile([128, BL], f32, tag="trg")
                nc.tensor.transpose(out=gp_ps[:kn, :], in_=gap_sb[:, ks:ks + kn],
                                    identity=ident_sb[:BL, :BL])
                nc.scalar.activation(out=gapT_sb[kk][:kn, :], in_=gp_ps[:kn, :],
                                     func=AF.Copy)

            # ================= Phase E: MLPs =================
            y1_sb = cpool.tile([128, 12 * BL], f32, tag="y1")
            for mi, (ms, mn) in enumerate(tiles(1500)):
                y_ps = psm.tile([128, BL], f32, tag="mlp")
                for kk, (ks, kn) in enumerate(FCH):
                    wt = wpool.tile([128, 128], f32, tag="wt")
                    nc.sync.dma_start(out=wt[:kn, :mn],
                                      in_=fcg1_w[ks:ks + kn, ms:ms + mn])
                    nc.tensor.matmul(out=y_ps[:mn, :], lhsT=wt[:kn, :mn],
                                     rhs=gmpT_sb[kk][:kn, :],
                                     start=(kk == 0), stop=False)
                for kk, (ks, kn) in enumerate(FCH):
                    wt = wpool.tile([128, 128], f32, tag="wt")
                    nc.sync.dma_start(out=wt[:kn, :mn],
                                      in_=fcg1_w[HID + ks:HID + ks + kn, ms:ms + mn])
                    nc.tensor.matmul(out=y_ps[:mn, :], lhsT=wt[:kn, :mn],
                                     rhs=gapT_sb[kk][:kn, :],
                                     start=False, stop=(kk == len(FCH) - 1))
                nc.scalar.activation(out=y1_sb[:mn, mi * BL:(mi + 1) * BL],
                                     in_=y_ps[:mn, :], func=AF.Relu,
                                     bias=fcg1b_sb[:mn, mi:mi + 1])

            xc0 = cpool.tile([128, BL], f32, tag="xc0")
            y2_ps = psm.tile([128, BL], f32, tag="mlp")
            kt2 = tiles(1500)
            for kk, (ks, kn) in enumerate(kt2):
                wt = wpool.tile([128, 128], f32, tag="wt")
                nc.sync.dma_start(out=wt[:kn, :], in_=fcg2_w[ks:ks + kn, :])
                nc.tensor.matmul(out=y2_ps[:], lhsT=wt[:kn, :],
                                 rhs=y1_sb[:kn, kk * BL:(kk + 1) * BL],
                                 start=(kk == 0), stop=(kk == len(kt2) - 1))
            nc.scalar.activation(out=xc0[:], in_=y2_ps[:], func=AF.Copy,
                                 bias=fcg2b_sb[:, 0:1])

            # ---- protein branch ----
            cvT_sb = cpool.tile([121, BL * 32], f32, tag="cvT")
            for b in range(BL):
                oh = pool.tile([128, 8 * 26], f32, tag="oh")
                nc.vector.tensor_tensor(
                    out=oh[:].rearrange("p (k c) -> p k c", c=26),
                    in0=iota26_sb[:].rearrange("p (k c) -> p k c", c=26),
                    in1=t_sb[:, b * 8:(b + 1) * 8].unsqueeze(2)
                        .to_broadcast([128, 8, 26]),
                    op=OP.is_equal)
                at_ps = psm.tile([26, 256], f32, tag="convA")
                for ic in range(8):
                    icn = min(128, 1000 - ic * 128)
                    nc.tensor.matmul(out=at_ps[:],
                                     lhsT=oh[:icn, ic * 26:(ic + 1) * 26],
                                     rhs=w2_sb[ic][:icn, :],
                                     start=(ic == 0), stop=(ic == 7))
                at_sb = pool.tile([26, 256], f32, tag="at_sb")
                nc.scalar.activation(out=at_sb[:], in_=at_ps[:], func=AF.Copy)
                cv_ps = psm.tile([121, 32], f32, tag="convC")
                for k in range(8):
                    nc.tensor.matmul(out=cv_ps[:], lhsT=emb_sb[:, k:k + 121],
                                     rhs=at_sb[:, k * 32:(k + 1) * 32],
                                     start=(k == 0), stop=(k == 7))
                nc.scalar.activation(out=cvT_sb[:, b * 32:(b + 1) * 32], in_=cv_ps[:],
                                     func=AF.Copy)
            xc1 = cpool.tile([128, BL], f32, tag="xc1")
            xt_ps = psm.tile([128, BL], f32, tag="mlp")
            for o in range(32):
                wt = wpool.tile([128, 128], f32, tag="wt")
                nc.sync.dma_start(out=wt[:121, :], in_=fcxt_w[o * 121:(o + 1) * 121, :])
                nc.tensor.matmul(
                    out=xt_ps[:], lhsT=wt[:121, :],
                    rhs=cvT_sb[:].rearrange("p (b o) -> p b o", o=32)[:, :, o],
                    start=(o == 0), stop=(o == 31))
            nc.scalar.activation(out=xc1[:], in_=xt_ps[:], func=AF.Copy,
                                 bias=fcxtb_sb[:, 0:1])

            # ---- head ----
            y3_sb = cpool.tile([128, 8 * BL], f32, tag="y3")
            for mi in range(8):
                y_ps = psm.tile([128, BL], f32, tag="mlp")
                for kk in range(2):
                    wt = wpool.tile([128, 128], f32, tag="wt")
                    nc.sync.dma_start(out=wt[:],
                                      in_=fc1_w[kk * 128:(kk + 1) * 128,
                                                mi * 128:(mi + 1) * 128])
                    rhs = xc0 if kk == 0 else xc1
                    nc.tensor.matmul(out=y_ps[:], lhsT=wt[:], rhs=rhs[:],
                                     start=(kk == 0), stop=(kk == 1))
                nc.scalar.activation(out=y3_sb[:, mi * BL:(mi + 1) * BL], in_=y_ps[:],
                                     func=AF.Relu, bias=fc1b_sb[:, mi:mi + 1])
            y4_sb = cpool.tile([128, 4 * BL], f32, tag="y4")
            for mi in range(4):
                y_ps = psm.tile([128, BL], f32, tag="mlp")
                for kk in range(8):
                    wt = wpool.tile([128, 128], f32, tag="wt")
                    nc.sync.dma_start(out=wt[:],
                                      in_=fc2_w[kk * 128:(kk + 1) * 128,
                                                mi * 128:(mi + 1) * 128])
                    nc.tensor.matmul(out=y_ps[:], lhsT=wt[:],
                                     rhs=y3_sb[:, kk * BL:(kk + 1) * BL],
                                     start=(kk == 0), stop=(kk == 7))
                nc.scalar.activation(out=y4_sb[:, mi * BL:(mi + 1) * BL], in_=y_ps[:],
                                     func=AF.Relu, bias=fc2b_sb[:, mi:mi + 1])
            o_ps = psm.tile([1, BL], f32, tag="mlp")
            for kk in range(4):
                wt = wpool.tile([128, 1], f32, tag="wto")
                nc.sync.dma_start(out=wt[:], in_=out_w[kk * 128:(kk + 1) * 128, :])
                nc.tensor.matmul(out=o_ps[:], lhsT=wt[:],
                                 rhs=y4_sb[:, kk * BL:(kk + 1) * BL],
                                 start=(kk == 0), stop=(kk == 3))
            o_sb = cpool.tile([1, BL], f32, tag="o_sb")
            nc.scalar.activation(out=o_sb[:], in_=o_ps[:], func=AF.Copy,
                                 bias=outb_sb[:, 0:1])
            nc.sync.dma_start(out=outp[:, 0], in_=o_sb[0, :])

    nc.finalize()
    return nc


def run(inp, n_cores=8, trace=False):
    from concourse.bass_utils import run_bass_kernel_spmd
    in_maps, cfg = host_prep(inp, n_cores)
    nc = build(cfg)
    res = run_bass_kernel_spmd(
        nc, in_maps, list(range(n_cores)), trace=trace,
        trace_cores=list(range(n_cores)) if trace else None)
    out = np.concatenate([res.results[c]["outp"] for c in range(n_cores)], 0)
    return out, res


_CACHED = {}


def kernel(**inputs):
    """Full-input entry point: shards across 8 NeuronCores internally."""
    n_cores = 8
    in_maps, cfg = host_prep(inputs, n_cores)
    key = (cfg["N"], cfg["T"], cfg["BL"], cfg["K"], cfg["GN"])
    nc = _CACHED.get(key)
    if nc is None:
        nc = build(cfg)
        _CACHED[key] = nc
    from concourse.bass_utils import run_bass_kernel_spmd
    res = run_bass_kernel_spmd(nc, in_maps, list(range(n_cores)))
    out = np.concatenate(
        [res.results[c]["outp"].reshape(-1, 1) for c in range(n_cores)], 0)
    return out.astype(np.float32)
